# revision 15
# baseline (speedup 1.0000x reference)
"""Single-head self-attention (B=8, S=2048, D=1024) on 8 TRN2 NeuronCores.

Data-parallel over batch: core b computes attention for x[b].
All compute in bf16 matmuls with fp32 PSUM accumulation; softmax in fp32.

Prologue: x streams fp32 over the in-order sync HWDGE queue in 256-row
half-chunks and is transposed ON THE PE (is_transpose matmuls against an
identity) -- XBAR DMA transposes serialize at ~4.5us/instruction on their
queue, the PE does a 128x128 tile in ~107ns.  Weights are SWDGE
fp32->bf16 casts on gpsimd with fences only BETWEEN weight tensors (the
fence bubble lands inside compute phases).  Softmax column sums use a
DVE add tree plus one small matmul per query chunk.
"""

import sys

sys.path.insert(0, "/opt/trn_rl_repo")

import numpy as np

B, S, D = 8, 2048, 1024
P = 128
SO = S // P  # 16 s-tiles
DO = D // P  # 8 d-tiles
IC = 512  # i-chunk (query chunk) width
NIC = S // IC  # 4
NF = D // 512  # 2 free-dim chunks for D-wide outputs
HC = 256  # x load half-chunk rows
NHC = S // HC  # 8

_CACHE = {}


def _emit_body(nc, tc, t):
    import concourse.mybir as mybir
    from concourse import masks

    F32 = mybir.dt.float32
    F32R = mybir.dt.float32r
    BF16 = mybir.dt.bfloat16
    Exp = mybir.ActivationFunctionType.Exp
    Ident = mybir.ActivationFunctionType.Identity

    const = tc.alloc_tile_pool(name="const", bufs=1)
    dram = tc.alloc_tile_pool(name="dram", bufs=1, space="DRAM")

    # ---- small loads on the scalar HWDGE queue
    bq_sb = const.tile([P, DO], F32, name="bq_sb")
    nc.scalar.dma_start(bq_sb[:], t["bq"].rearrange("(eo ei) -> ei eo", ei=P))
    bk_sb = const.tile([P, DO], F32, name="bk_sb")
    nc.scalar.dma_start(bk_sb[:], t["bk"].rearrange("(eo ei) -> ei eo", ei=P))
    bv_row = const.tile([1, D], F32, name="bv_row")
    nc.scalar.dma_start(bv_row[:], t["bv"].rearrange("(a d) -> a d", a=1))
    bo_row = const.tile([1, D], F32, name="bo_row")
    nc.scalar.dma_start(bo_row[:], t["bo"].rearrange("(a d) -> a d", a=1))
    ones_col = const.tile([P, 1], BF16, name="ones_col")
    nc.vector.memset(ones_col[:], 1.0)
    ident = const.tile([P, P], F32, name="ident")
    masks.make_identity(nc, ident[:])
    ident_bf = const.tile([P, P], BF16, name="ident_bf")
    masks.make_identity(nc, ident_bf[:])

    # ---- weights: fp32->bf16 SWDGE casts into SBUF.  Fences only between
    # weight tensors so in-flight round-robin never dilutes an earlier,
    # sooner-needed weight.
    wqkv = tc.alloc_tile_pool(name="wqkv", bufs=1)
    W_sb = {
        n: wqkv.tile([P, DO, D], BF16, name=f"{n}_sb") for n in ("Wq", "Wk", "Wv")
    }
    fence = const.tile([1, 32], BF16, name="fence")

    def load_w(name):
        for h in range(2):
            nc.gpsimd.dma_start(
                W_sb[name][:, :, h * 512 : (h + 1) * 512],
                t[name].rearrange("(ko ki) e -> ki ko e", ki=P)[
                    :, :, h * 512 : (h + 1) * 512
                ],
            )

    def fence_w(name):
        # tiny SWDGE read spanning both column-half writes of W_sb[name]
        nc.gpsimd.dma_start(fence[:], W_sb[name][0:1, 7, 496:528])

    load_w("Wq")
    fence_w("Wq")
    load_w("Wk")
    fence_w("Wk")
    load_w("Wv")

    # ---- x: plain fp32 loads on the sync HWDGE queue (in-order, no casts)
    xt_pool = tc.alloc_tile_pool(name="xt_pool", bufs=1)
    xT = xt_pool.tile([P, DO, S], BF16, name="xT")  # [d_inner, d_outer, s]
    xstage = tc.alloc_tile_pool(name="xstage", bufs=2)
    xbf_pool = tc.alloc_tile_pool(name="xbf_pool", bufs=1)
    xbf_st = xbf_pool.tile([P, D], BF16, name="xbf_st")
    x_r = t["x"].rearrange("(hc si p) d -> hc p si d", p=P, si=HC // P)
    stg = []
    for hc in range(NHC):
        st = xstage.tile([P, HC // P, D], F32, tag="xs", name=f"xs{hc}")
        nc.sync.dma_start(st[:], x_r[hc])
        stg.append(st)

    # ---- persistent activations
    bv_bcast = const.tile([P, D], F32, name="bv_bcast")
    QT = const.tile([P, DO, S], BF16, name="QT")  # [e_i, e_o, s]
    KT = const.tile([P, DO, S], BF16, name="KT")
    V = const.tile([P, SO, D], BF16, name="V")  # [s_i, s_o, e]
    recip_sb = const.tile([P, SO], F32, name="recip_sb")

    # bias row broadcast on the (otherwise idle) gpsimd engine
    nc.gpsimd.partition_broadcast(bv_bcast[:], bv_row[:])

    with tc.tile_pool(name="ppsum", bufs=4, space="PSUM") as ppsum, \
         tc.tile_pool(name="tpsum", bufs=2, space="PSUM") as tpsum, \
         tc.tile_pool(name="tpsum_bf", bufs=2, space="PSUM") as tpsum_bf:
        def transpose_hc(hc):
            # PE transpose of one 256-row half-chunk into xT
            for do in range(DO):
                ps = tpsum.tile([P, HC], F32, tag="xt", name="tps")
                for si in range(HC // P):
                    nc.tensor.matmul(
                        ps[:, si * P : (si + 1) * P],
                        stg[hc][:, si, do * P : (do + 1) * P],
                        ident[:],
                        start=True,
                        stop=True,
                        is_transpose=True,
                        skip_group_check=True,
                    )
                nc.vector.tensor_copy(
                    xT[:, do, hc * HC : (hc + 1) * HC], ps[:]
                )

        def transpose_hc_bf(hc):
            # pre-cast on DVE: the scalar queue is busy with proj
            # activations here and would serialize behind them
            for si in range(HC // P):
                nc.vector.tensor_copy(xbf_st[:], stg[hc][:, si, :])
                for do in range(DO):
                    ps = tpsum_bf.tile([P, P], BF16, tag="xtb", name="tpsb")
                    nc.tensor.matmul(
                        ps[:],
                        xbf_st[:, do * P : (do + 1) * P],
                        ident_bf[:],
                        start=True,
                        stop=True,
                        is_transpose=True,
                        skip_group_check=True,
                    )
                    nc.vector.tensor_copy(
                        xT[:, do, hc * HC + si * P : hc * HC + (si + 1) * P],
                        ps[:],
                    )

        def proj_qk(Wn, b_sb, OUT, sc):
            for eo in range(DO):
                ps = ppsum.tile([P, 512], F32, tag="proj", name="pp")
                for k in range(DO):
                    nc.tensor.matmul(
                        ps[:],
                        W_sb[Wn][:, k, eo * P : (eo + 1) * P],
                        xT[:, k, sc * 512 : (sc + 1) * 512],
                        start=(k == 0),
                        stop=(k == DO - 1),
                    )
                nc.scalar.activation(
                    OUT[:, eo, sc * 512 : (sc + 1) * 512],
                    ps[:],
                    Ident,
                    bias=b_sb[:, eo : eo + 1],
                )

        # transpose half-chunks just-in-time: hc0-3 fill the tensor-idle
        # head while Wq streams in; hc4-7 interleave between Q chunks
        transpose_hc(0)
        transpose_hc(1)
        transpose_hc(2)
        transpose_hc(3)
        proj_qk("Wq", bq_sb, QT, 0)
        transpose_hc_bf(4)
        transpose_hc_bf(5)
        proj_qk("Wq", bq_sb, QT, 1)
        transpose_hc_bf(6)
        transpose_hc_bf(7)
        proj_qk("Wq", bq_sb, QT, 2)
        proj_qk("Wq", bq_sb, QT, 3)
        for sc in range(NIC):
            proj_qk("Wk", bk_sb, KT, sc)
        for so in range(SO):
            pss = [
                ppsum.tile([P, 512], F32, tag="proj", name=f"pv{fc}")
                for fc in range(NF)
            ]
            for k in range(DO):
                for fc in range(NF):
                    nc.tensor.matmul(
                        pss[fc][:],
                        xT[:, k, so * P : (so + 1) * P],
                        W_sb["Wv"][:, k, fc * 512 : (fc + 1) * 512],
                        start=(k == 0),
                        stop=(k == DO - 1),
                    )
            for fc in range(NF):
                nc.vector.tensor_add(
                    V[:, so, fc * 512 : (fc + 1) * 512],
                    pss[fc][:],
                    bv_bcast[:, fc * 512 : (fc + 1) * 512],
                )

    # projections done: reclaim x staging, xT, and Wq/Wk/Wv space
    xbf_pool.release()
    xstage.release()
    xt_pool.release()
    wqkv.release()

    late = tc.alloc_tile_pool(name="late", bufs=1)
    YT = late.tile([P, DO, S], BF16, name="YT")  # [e_i, e_o, i]
    Wo_sb = late.tile([P, DO, D], BF16, name="Wo_sb")
    nc.gpsimd.dma_start(
        Wo_sb[:], t["Wo"].rearrange("(ko ki) e -> ki ko e", ki=P)
    )
    bo_bcast = late.tile([P, D], F32, name="bo_bcast")
    nc.gpsimd.partition_broadcast(bo_bcast[:], bo_row[:])

    cs_dram = dram.tile([S], F32, name="cs_dram")
    cs_dram_2d = cs_dram.rearrange("(a s) -> a s", a=1)

    # ---- attention: per query-chunk of 512
    inv_sqrt_d = float(1.0 / np.sqrt(D))
    with tc.tile_pool(name="epool", bufs=2) as epool, \
         tc.tile_pool(name="tpool", bufs=1) as tpool, \
         tc.tile_pool(name="csb_pool", bufs=2) as csb_pool, \
         tc.tile_pool(name="spsum", bufs=3, space="PSUM") as spsum, \
         tc.tile_pool(name="cpsum", bufs=1, space="PSUM") as cpsum, \
         tc.tile_pool(name="ypsum", bufs=4, space="PSUM") as ypsum:
        Tt = [tpool.tile([P, IC], F32, name=f"T{i}") for i in range(4)]
        Tb = tpool.tile([P, IC], BF16, name="Tb")
        for ic in range(NIC):
            isl = slice(ic * IC, (ic + 1) * IC)
            # E = exp(S^T/sqrt(D)) in [j_inner, j_outer, i] layout.  The
            # colsum add-tree is interleaved so each DVE add only waits on
            # E tiles that already exist; all inputs of an add share dtype.
            E = epool.tile([P, SO, IC], BF16, tag="E", name="E")

            def Es(j):
                return E[:, j, :]

            add = nc.vector.tensor_add
            tree = {
                1: [(Tt[0], Es(0), Es(1))],
                3: [(Tt[1], Es(2), Es(3)), (Tt[0], Tt[0][:], Tt[1][:])],
                5: [(Tt[1], Es(4), Es(5))],
                7: [(Tt[2], Es(6), Es(7)), (Tt[1], Tt[1][:], Tt[2][:]),
                    (Tt[0], Tt[0][:], Tt[1][:])],
                9: [(Tt[1], Es(8), Es(9))],
                11: [(Tt[2], Es(10), Es(11)), (Tt[1], Tt[1][:], Tt[2][:])],
                13: [(Tt[2], Es(12), Es(13))],
                15: [(Tt[3], Es(14), Es(15)), (Tt[2], Tt[2][:], Tt[3][:]),
                     (Tt[1], Tt[1][:], Tt[2][:]), (Tb, Tt[0][:], Tt[1][:])],
            }
            for jt in range(SO):
                ps = spsum.tile([P, IC], F32, tag="S", name="sps")
                for k in range(DO):
                    nc.tensor.matmul(
                        ps[:],
                        KT[:, k, jt * P : (jt + 1) * P],
                        QT[:, k, isl],
                        start=(k == 0),
                        stop=(k == DO - 1),
                    )
                nc.scalar.activation(E[:, jt, :], ps[:], Exp, scale=inv_sqrt_d)
                for out_t, a, b in tree.get(jt, ()):
                    add(out_t[:], a, b)
            # softmax denominators: single ones-matmul over the tree sum
            cs = cpsum.tile([1, IC], F32, tag="cs", name="cs")
            nc.tensor.matmul(cs[:], ones_col[:], Tb[:], start=True, stop=True)
            csb = csb_pool.tile([1, IC], F32, tag="csb", name="csb")
            nc.vector.tensor_copy(csb[:], cs[:])
            nc.sync.dma_start(cs_dram_2d[:, isl], csb[:])
            # Y^T (unnormalized): lhsT = V tile [j, e-tile], rhs = E [j, i]
            for eo in range(DO):
                py = ypsum.tile([P, IC], F32, tag="Y", name="yps")
                for jt in range(SO):
                    nc.tensor.matmul(
                        py[:],
                        V[:, jt, eo * P : (eo + 1) * P],
                        E[:, jt, :],
                        start=(jt == 0),
                        stop=(jt == SO - 1),
                    )
                nc.vector.tensor_copy(YT[:, eo, isl], py[:])

    # reshape colsum [S] in DRAM -> [128, SO] (per-partition for output)
    nc.sync.dma_start(recip_sb[:], cs_dram.rearrange("(io ii) -> ii io", ii=P))
    nc.vector.reciprocal(recip_sb[:], recip_sb[:])

    # ---- output projection: out = (Y^T.T @ Wo) * recip + bo
    out_r = t["out"].rearrange("(so si) f -> si so f", si=P)
    with tc.tile_pool(name="opool", bufs=3) as opool, \
         tc.tile_pool(name="opsum", bufs=4, space="PSUM") as opsum:
        for it in range(SO):
            pss = [
                opsum.tile([P, 512], F32, tag="O", name=f"po{fc}")
                for fc in range(NF)
            ]
            for k in range(DO):
                for fc in range(NF):
                    nc.tensor.matmul(
                        pss[fc][:],
                        YT[:, k, it * P : (it + 1) * P],
                        Wo_sb[:, k, fc * 512 : (fc + 1) * 512],
                        start=(k == 0),
                        stop=(k == DO - 1),
                    )
            o_sb = opool.tile([P, D], F32, tag="osb", name="o_sb")
            for fc in range(NF):
                fsl = slice(fc * 512, (fc + 1) * 512)
                # fused out = (psum * recip) + bo in one DVE pass
                nc.vector.scalar_tensor_tensor(
                    o_sb[:, fsl],
                    pss[fc][:],
                    recip_sb[:, it : it + 1],
                    bo_bcast[:, fsl],
                    mybir.AluOpType.mult,
                    mybir.AluOpType.add,
                )
                q = nc.sync if fc == 0 else nc.scalar
                q.dma_start(out_r[:, it, fsl], o_sb[:, fsl])

    late.release()
    dram.release()
    const.release()


def _build():
    if "nc" in _CACHE:
        return _CACHE["nc"]
    import concourse.tile as tile
    import concourse.mybir as mybir
    from concourse import bacc

    nc = bacc.Bacc("TRN2", target_bir_lowering=False, debug=False, num_devices=8)
    F32 = mybir.dt.float32
    t = {}
    t["x"] = nc.dram_tensor("x", [S, D], F32, kind="ExternalInput").ap()
    for name in ("Wq", "Wk", "Wv", "Wo"):
        t[name] = nc.dram_tensor(name, [D, D], F32, kind="ExternalInput").ap()
    for name in ("bq", "bk", "bv", "bo"):
        t[name] = nc.dram_tensor(name, [D], F32, kind="ExternalInput").ap()
    t["out"] = nc.dram_tensor("out", [S, D], F32, kind="ExternalOutput").ap()

    with tile.TileContext(nc) as tc:
        _emit_body(nc, tc, t)
    nc.compile()
    _CACHE["nc"] = nc
    return nc


def kernel(x, Wq, bq, Wk, bk, Wv, bv, Wo, bo, _trace=False):
    from concourse.bass_utils import run_bass_kernel_spmd

    nc = _build()
    x = np.ascontiguousarray(np.asarray(x, dtype=np.float32))
    shared = {
        "Wq": np.ascontiguousarray(np.asarray(Wq, dtype=np.float32)),
        "Wk": np.ascontiguousarray(np.asarray(Wk, dtype=np.float32)),
        "Wv": np.ascontiguousarray(np.asarray(Wv, dtype=np.float32)),
        "Wo": np.ascontiguousarray(np.asarray(Wo, dtype=np.float32)),
        "bq": np.ascontiguousarray(np.asarray(bq, dtype=np.float32)),
        "bk": np.ascontiguousarray(np.asarray(bk, dtype=np.float32)),
        "bv": np.ascontiguousarray(np.asarray(bv, dtype=np.float32)),
        "bo": np.ascontiguousarray(np.asarray(bo, dtype=np.float32)),
    }
    in_maps = [{"x": x[b], **shared} for b in range(B)]
    res = run_bass_kernel_spmd(
        nc, in_maps, core_ids=list(range(B)), trace=_trace
    )
    out = np.stack([r["out"] for r in res.results], axis=0)
    if _trace:
        return out, res
    return out
